# revision 1
# baseline (speedup 1.0000x reference)
"""Trainium2 8-core fused attention kernel (QKV proj + RMSNorm + RoPE + SDPA + out proj).

Sharding: tensor-parallel over heads. Each of the 8 cores computes 2 of the 16
heads end-to-end (QKV projection with its Wqkv column shard, per-head RMSNorm +
RoPE, full softmax attention), then an AllToAll redistributes the per-head
attention outputs so every core holds all 1024 attention channels for 1/8 of
the tokens and applies the full Wout to its token shard.

v2: AV matmul flipped (probs stationary, [v|1] moving) so each PE instruction
streams 65 rows instead of 512 for the same MACs; softmax denominator rides the
ones column and lands per-partition, making the normalize a free-dim broadcast;
per-batch AllToAll fires as soon as that batch's attention drains so the
collective and the output projection overlap batch-1 attention.

Self-contained: hardcodes all shapes from the problem spec.
"""
import os
import sys
import types

import numpy as np
import ml_dtypes

sys.path.insert(0, "/opt/trn_rl_repo")

from concourse import bass, bacc, tile, mybir  # noqa: E402
from concourse.bass_utils import run_bass_kernel_spmd  # noqa: E402
from concourse.masks import make_identity  # noqa: E402

B, N, C, H, D = 2, 4096, 1024, 16, 64
NCORES = 8
TOK = B * N            # 8192 global tokens
NB = N // 128          # 32 token tiles per batch
NMACRO = N // 256      # 16 macro tiles (256 tok) per batch
QTILE = 512
NQT = N // QTILE       # 8 q tiles per batch
KC = N // 128          # 32 key chunks per batch
SHARD = TOK // NCORES  # 1024 tokens per core shard
EPS = 1e-6

F32 = mybir.dt.float32
BF16 = mybir.dt.bfloat16
U16 = mybir.dt.uint16
ALU = mybir.AluOpType
ACTF = mybir.ActivationFunctionType

# Schraudolph exp-via-bits for the DVE: bf16 bitpattern of exp(0.125*s) is
# approx round(A*s + B) as uint16 (error <= +-4.2%, rms 1.8%). Batch-1 head-1
# probs use this (25% of elements) to take load off the Scalar engine; the
# approximation bias cancels between softmax numerator and denominator.
SCHRAUD_A = 16.0 / np.log(2.0)
SCHRAUD_B = 16248.75

_CACHE = {}
_LAST_RESULT = None


def _install_profile_shim():
    """trn_boot skips the NTFF hook when antenv.axon_hooks is missing; supply it."""
    try:
        import antenv
        if getattr(antenv, "axon_hooks", None) is not None:
            return
        from trn_agent_boot.trn_boot import _ntff_profile_via_ctypes
        hook = _ntff_profile_via_ctypes("/opt/axon/libaxon_pjrt.so")
        if hook is None:
            return
        mod = types.ModuleType("antenv.axon_hooks")
        state = {"hook": hook}
        mod.get_axon_ntff_profile_hook = lambda: state["hook"]
        mod.set_axon_ntff_profile_hook = lambda h: state.__setitem__("hook", h)
        sys.modules["antenv.axon_hooks"] = mod
        antenv.axon_hooks = mod
    except Exception:
        pass


def _build_graph():
    nc = bacc.Bacc("TRN2", target_bir_lowering=False, debug=False,
                   enable_asserts=True, num_devices=NCORES)

    hsT_d = nc.dram_tensor("hsT", [C, TOK], BF16, kind="ExternalInput")
    wqkv_d = nc.dram_tensor("wqkv", [C, 384], BF16, kind="ExternalInput")
    trigc_d = nc.dram_tensor("trigc", [N, 256], BF16, kind="ExternalInput")
    trigs_d = nc.dram_tensor("trigs", [N, 256], BF16, kind="ExternalInput")
    wout_d = nc.dram_tensor("wout", [C, C], BF16, kind="ExternalInput")
    out_d = nc.dram_tensor("out", [SHARD, C], F32, kind="ExternalOutput")

    with tile.TileContext(nc) as tc:
        with tc.tile_pool(name="const", bufs=1) as constp, \
             tc.tile_pool(name="dram", bufs=1, space="DRAM") as dram:
            # resident weights
            # per-chunk loads so the first QKV chain starts as soon as chunk 0
            # lands instead of waiting for the whole 768KB tensor
            wqkv_sb = constp.tile([128, 8, 384], BF16)
            for cc in range(8):
                nc.sync.dma_start(
                    wqkv_sb[:, cc, :],
                    wqkv_d.ap().rearrange("(a p) n -> p a n", p=128)[:, cc, :])
            ident = constp.tile([128, 128], BF16)
            make_identity(nc, ident[:])

            a2a_in = [dram.tile([NCORES, 128, SHARD // 2], BF16,
                                  name=f"a2a_in{h}", tag=f"a2a_in{h}") for h in range(2)]
            a2a_out = [dram.tile([NCORES, 128, SHARD // 2], BF16,
                                   name=f"a2a_out{h}", tag=f"a2a_out{h}") for h in range(2)]

            with tc.tile_pool(name="batch", bufs=1) as bp, \
                 tc.tile_pool(name="work", bufs=3) as wp, \
                 tc.tile_pool(name="probsp", bufs=6) as pp, \
                 tc.tile_pool(name="pssc", bufs=2, space="PSUM") as pssc, \
                 tc.tile_pool(name="psav", bufs=1, space="PSUM") as psav, \
                 tc.tile_pool(name="psT", bufs=2, space="PSUM") as psT:

                qT = [bp.tile([128, N], BF16, name=f"qT{b}", tag=f"qT{b}") for b in range(B)]
                kT = [bp.tile([128, N], BF16, name=f"kT{b}", tag=f"kT{b}") for b in range(B)]
                vsb = [bp.tile([128, NB, 2, 65], BF16, name=f"v{b}", tag=f"v{b}")
                       for b in range(B)]
                # atn2[b][h]: col block pq holds q-chunks (2pq, 2pq+1) of head
                # h transposed: rows 0:64 = chunk 2pq [d, q], rows 64:128 =
                # chunk 2pq+1 (lets the drain transpose [128,128] pairs)
                atn2 = [[bp.tile([128, N // 2], BF16, name=f"at{b}{h}",
                                 tag=f"at{b}{h}") for h in range(2)]
                        for b in range(B)]
                for b in range(B):
                    nc.vector.memset(vsb[b][:, :, :, 64:65], 1.0)

                # PE p-state warmup while the first hsT tiles stream in: the
                # tensor engine ramps to full clock after ~3us of activity.
                warm = psT.tile([128, 128], F32, name="warm", tag="pst")
                for _ in range(180):
                    nc.tensor.matmul(warm[:], lhsT=ident[:], rhs=ident[:],
                                     start=True, stop=True)

                pending_drains = []

                def flush_drains():
                    while pending_drains:
                        pending_drains.pop(0)()

                # ---------------- Stage A: QKV + RMSNorm + RoPE + transposes ----
                def emit_A(b, mt, sp):
                    flush_drains()
                    hs_t = []
                    for cc in range(8):
                        t = sp.tile([128, 256], BF16, name=f"hs{cc}", tag=f"hs{cc}")
                        nc.sync.dma_start(
                            t[:], hsT_d.ap()[cc * 128:(cc + 1) * 128,
                                             b * N + mt * 256: b * N + (mt + 1) * 256])
                        hs_t.append(t)
                    trigC = sp.tile([128, 2, 256], BF16, name="trigC", tag="trigC")
                    trigS = sp.tile([128, 2, 256], BF16, name="trigS", tag="trigS")
                    for dst, dt_ in ((trigC, trigc_d), (trigS, trigs_d)):
                        nc.sync.dma_start(
                            dst[:], dt_.ap()[mt * 256:(mt + 1) * 256, :]
                            .rearrange("(s p) d -> p s d", p=128))

                    # pass 1: both sub-tiles' QKV chains back-to-back so PE is
                    # not parked behind the serial DVE RMSNorm chain
                    cpeng = nc.scalar if (b == 0 and mt < NMACRO // 2) else nc.vector
                    qk_sbs = []
                    for sub in range(2):
                        tt = mt * 2 + sub  # token tile index within batch
                        # q+k chain in bank 0, v chain in bank 1: alternating
                        # banks lets the accumulating writes pipeline
                        ps_qkv = pssc.tile([128, 1024], F32, name="ps_qkv", tag="pssc")
                        for cc in range(8):
                            lhs = hs_t[cc][:, sub * 128:(sub + 1) * 128]
                            nc.tensor.matmul(
                                ps_qkv[:, 0:256], lhsT=lhs,
                                rhs=wqkv_sb[:, cc, 0:256],
                                start=(cc == 0), stop=(cc == 7))
                            nc.tensor.matmul(
                                ps_qkv[:, 512:640], lhsT=lhs,
                                rhs=wqkv_sb[:, cc, 256:384],
                                start=(cc == 0), stop=(cc == 7))

                        # q/k block to SBUF; in the batch-0 prefix ACT is idle, so
                        # route the copies there (Copy needs no ACT table switch)
                        qk_sb = wp.tile([128, 256], F32, name="qk_sb", tag="qk_sb",
                                        bufs=4)
                        if cpeng is nc.scalar:
                            nc.scalar.copy(qk_sb[:], ps_qkv[:, 0:256])
                        else:
                            nc.vector.tensor_copy(qk_sb[:], ps_qkv[:, 0:256])
                        nc.vector.tensor_copy(
                            vsb[b][:, tt, :, 0:64],
                            ps_qkv[:, 512:640].rearrange("p (h d) -> p h d", h=2))
                        qk_sbs.append(qk_sb)

                    # pass 2: RMSNorm + RoPE chains (DVE), pass 3: transposes
                    d_bfs = []
                    for sub in range(2):
                        qk_sb = qk_sbs[sub]
                        # sumsq for (q h0, q h1, k h0, k h1) -> [128, 4]
                        sq = wp.tile([128, 256], F32, name="sq", tag="sq", bufs=4)
                        ssq4 = wp.tile([128, 4], F32, name="ssq4", tag="ssq4",
                                       bufs=4)
                        nc.vector.tensor_mul(sq[:], qk_sb[:], qk_sb[:])
                        nc.vector.tensor_reduce(
                            ssq4[:], sq[:].rearrange("p (a e) -> p a e", a=4),
                            axis=mybir.AxisListType.X, op=ALU.add)
                        # rinv = 8/sqrt(ssq): bit-trick seed + 1 Newton step
                        # (the /64 mean and *8 fold together; eps negligible here)
                        yv = wp.tile([128, 4], F32, name="yv", tag="yv")
                        with nc.allow_low_precision(reason="rsqrt newton seed"):
                            nc.vector.tensor_scalar(
                                out=yv[:].bitcast(mybir.dt.int32),
                                in0=ssq4[:].bitcast(mybir.dt.int32),
                                scalar1=1, scalar2=None, op0=ALU.arith_shift_right)
                            nc.vector.tensor_scalar(
                                out=yv[:].bitcast(mybir.dt.int32),
                                in0=yv[:].bitcast(mybir.dt.int32),
                                scalar1=-1, scalar2=0x5F3759DF,
                                op0=ALU.mult, op1=ALU.add)
                        tn = wp.tile([128, 4], F32, name="tn", tag="tn")
                        nc.vector.tensor_mul(tn[:], yv[:], yv[:])
                        nc.vector.tensor_mul(tn[:], tn[:], ssq4[:])
                        nc.vector.tensor_scalar(out=tn[:], in0=tn[:],
                                                scalar1=-4.0, scalar2=12.0,
                                                op0=ALU.mult, op1=ALU.add)
                        nc.vector.tensor_mul(yv[:], yv[:], tn[:])
                        # normalize all 4 groups at once (free-dim broadcast of rinv)
                        qn2 = wp.tile([128, 256], F32, name="qn2", tag="qn2", bufs=3)
                        nc.vector.tensor_tensor(
                            out=qn2[:].rearrange("p (a e) -> p a e", a=4),
                            in0=qk_sb[:].rearrange("p (a e) -> p a e", a=4),
                            in1=yv[:].unsqueeze(2).broadcast_to([128, 4, 64]),
                            op=ALU.mult)
                        d_qk = wp.tile([128, 256], F32, name="d_qk", tag="d_qk", bufs=3)
                        nc.vector.tensor_mul(d_qk[:], qn2[:], trigC[:, sub, :])
                        trot = wp.tile([128, 256], F32, name="trot", tag="trot", bufs=3)
                        v4 = qn2[:].rearrange("p (a e) -> p a e", a=8)
                        s4 = trigS[:, sub, :].rearrange("p (a e) -> p a e", a=8)
                        t4 = trot[:].rearrange("p (a e) -> p a e", a=8)
                        nc.vector.tensor_mul(t4[:, 0:8:2, :], v4[:, 1:8:2, :],
                                             s4[:, 0:8:2, :])
                        nc.vector.tensor_mul(t4[:, 1:8:2, :], v4[:, 0:8:2, :],
                                             s4[:, 1:8:2, :])
                        d_bf = wp.tile([128, 256], BF16, name="d_bf", tag="d_bf", bufs=4)
                        nc.vector.tensor_add(d_bf[:], d_qk[:], trot[:])
                        d_bfs.append(d_bf)
                    # transposes deferred to the next emission unit's flush
                    # point: their qT/kT outputs aren't read for several
                    # qtiles, and deferring keeps PE off the DVE chain's tail
                    def a_transposes(b=b, mt=mt, d_bfs=d_bfs, cpeng=cpeng):
                        for sub in range(2):
                            tt = mt * 2 + sub
                            for half, dstname in ((0, "q"), (1, "k")):
                                ps_t = psT.tile([128, 128], BF16, name="ps_t",
                                                tag="pst")
                                nc.tensor.transpose(
                                    ps_t[:],
                                    d_bfs[sub][:, half * 128:(half + 1) * 128],
                                    ident[:])
                                dst = qT[b] if dstname == "q" else kT[b]
                                if cpeng is nc.scalar:
                                    nc.scalar.copy(
                                        dst[:, tt * 128:(tt + 1) * 128], ps_t[:])
                                else:
                                    nc.vector.tensor_copy(
                                        dst[:, tt * 128:(tt + 1) * 128], ps_t[:])
                    pending_drains.append(a_transposes)

                # ---------------- Stage B: attention --------------------------
                # Flipped AV: probs chunk [128kc, 128q] stationary, [v|1] moving.
                # at_ps[:, u, 0:65] (u = hh*4+j) accumulates [128q, 64d | denom].

                def emit_B(b, qt, kc_lo=0, kc_hi=KC - 1, acc=None, filler=None):
                    at_ps = psav.tile([128, 8, 128], F32, name="at_ps", tag="psav",
                                      bufs=1)
                    prevs = []

                    def av_half(pkc, ppr, hh):
                        # start=True clears accumulate bits for the WHOLE psum
                        # bank, so only the first group per bank (u=0, u=4) may
                        # set it; the rest begin with start=False on the
                        # freshly-cleared bank (overwrite-where-bit-unset).
                        for j in range(4):
                            u = hh * 4 + j
                            nc.tensor.matmul(
                                at_ps[:, u, 0:65],
                                lhsT=ppr[:, hh * QTILE + j * 128:
                                         hh * QTILE + (j + 1) * 128],
                                rhs=vsb[b][:, pkc, hh, :],
                                start=(pkc == kc_lo and j == 0),
                                stop=(pkc == kc_hi),
                                skip_group_check=(j != 0))

                    def scores(kc, hh):
                        nc.tensor.matmul(
                            ps_s[:, hh * QTILE:(hh + 1) * QTILE],
                            lhsT=kT[b][64 * hh:64 * (hh + 1),
                                       kc * 128:(kc + 1) * 128],
                            rhs=qT[b][64 * hh:64 * (hh + 1),
                                      qt * QTILE:(qt + 1) * QTILE],
                            start=True, stop=True)

                    for kc in range(kc_lo, kc_hi + 1):
                        ps_s = pssc.tile([128, 2 * QTILE], F32, name="ps_s", tag="pssc")
                        scores(kc, 0)
                        scores(kc, 1)
                        pr = pp.tile([128, 2 * QTILE], BF16, name="pr", tag="pr",
                                     bufs=6)
                        if b == 1:
                            # head 0 on ACT, head 1 via DVE Schraudolph
                            # (different psum banks -> legal parallel access)
                            nc.scalar.activation(pr[:, 0:QTILE], ps_s[:, 0:QTILE],
                                                 ACTF.Exp, bias=0.0, scale=0.125)
                            with nc.allow_low_precision(reason="schraudolph exp"):
                                nc.vector.tensor_scalar(
                                    out=pr[:, QTILE:2 * QTILE].bitcast(U16),
                                    in0=ps_s[:, QTILE:2 * QTILE],
                                    scalar1=float(SCHRAUD_A),
                                    scalar2=float(SCHRAUD_B),
                                    op0=ALU.mult, op1=ALU.add)
                        elif kc == kc_hi:
                            # split the segment's last exp so AV h0 (and the
                            # drain behind it) starts after the first half
                            # instead of waiting out the full-tile latency
                            nc.scalar.activation(pr[:, 0:QTILE], ps_s[:, 0:QTILE],
                                                 ACTF.Exp, bias=0.0, scale=0.125)
                            nc.scalar.activation(pr[:, QTILE:2 * QTILE],
                                                 ps_s[:, QTILE:2 * QTILE],
                                                 ACTF.Exp, bias=0.0, scale=0.125)
                        else:
                            nc.scalar.activation(pr[:], ps_s[:], ACTF.Exp,
                                                 bias=0.0, scale=0.125)
                        if kc == kc_lo + 2:
                            flush_drains()
                        if len(prevs) == 1:
                            ppkc, pppr = prevs.pop(0)
                            av_half(ppkc, pppr, 0)
                            av_half(ppkc, pppr, 1)
                            if filler is not None:
                                filler(kc)
                        prevs.append((kc, pr))

                    # final kc: interleave each accumulator's drain (or partial
                    # save) right behind its stop-matmul so the DVE overlaps
                    # the remaining AV matmuls instead of queueing after them
                    pkc, ppr = prevs.pop(0)
                    partial = kc_hi < KC - 1
                    dsbs = []
                    if not partial:
                        for p in range(4):
                            dsb = wp.tile([128, 128], BF16, name="dsb", tag="dsb",
                                          bufs=8)
                            dsbs.append(dsb)

                    def drain1_u(u):
                        if partial:
                            nc.vector.tensor_copy(acc[:, u, :], at_ps[:, u, 0:65])
                            return
                        if acc is not None and kc_lo > 0:
                            tot = wp.tile([128, 65], F32, name="tot", tag="tot",
                                          bufs=4)
                            nc.vector.tensor_add(tot[:], at_ps[:, u, 0:65],
                                                 acc[:, u, :])
                            src = tot[:]
                        else:
                            src = at_ps[:, u, 0:65]
                        rcp = wp.tile([128, 1], F32, name="rcp", tag="rcp", bufs=16)
                        nc.vector.reciprocal_approx_fast(out=rcp[:],
                                                         in_=src[:, 64:65])
                        nc.vector.tensor_tensor(
                            out=dsbs[u // 2][:, (u % 2) * 64:(u % 2) * 64 + 64],
                            in0=src[:, 0:64],
                            in1=rcp[:].broadcast_to([128, 64]), op=ALU.mult)

                    for hh in range(2):
                        for j in range(4):
                            u = hh * 4 + j
                            nc.tensor.matmul(
                                at_ps[:, u, 0:65],
                                lhsT=ppr[:, hh * QTILE + j * 128:
                                         hh * QTILE + (j + 1) * 128],
                                rhs=vsb[b][:, pkc, hh, :],
                                start=(pkc == kc_lo and j == 0),
                                stop=True, skip_group_check=(j != 0))
                            drain1_u(u)
                    if partial:
                        return

                    # drain part 2 (deferred, PE+DVE): transpose pairs into
                    # atn2, then stage this qtile's a2a block (dest core == qt)
                    def drain2(b=b, qt=qt, dsbs=dsbs):
                        for p in range(4):
                            hh, lp = divmod(p, 2)
                            pq = 2 * qt + lp
                            ps_t = psT.tile([128, 128], BF16, name="ps_tb",
                                            tag="pst")
                            nc.tensor.transpose(ps_t[:], dsbs[p][:], ident[:])
                            # in phase 3 the Scalar engine has slack (exp is
                            # half-offloaded) while the DVE queue is hot
                            if b == 1:
                                nc.scalar.copy(
                                    atn2[b][hh][:, pq * 128:(pq + 1) * 128],
                                    ps_t[:])
                            else:
                                nc.vector.tensor_copy(
                                    atn2[b][hh][:, pq * 128:(pq + 1) * 128],
                                    ps_t[:])
                        for hh in range(2):
                            for lp in range(2):
                                pq = 2 * qt + lp
                                src = atn2[b][hh][:, pq * 128:(pq + 1) * 128]
                                for rh in range(2):
                                    nc.sync.dma_start(
                                        a2a_in[b][qt, hh * 64:(hh + 1) * 64,
                                                  lp * 256 + rh * 128:
                                                  lp * 256 + (rh + 1) * 128],
                                        src[rh * 64:(rh + 1) * 64, :])
                    pending_drains.append(drain2)

                # ---- phases 1+2: batch-0 attention, all of stage A ----------
                with tc.tile_pool(name="stream", bufs=6) as sp, \
                     tc.tile_pool(name="accp", bufs=1) as accp:
                    at_acc = [accp.tile([128, 8, 65], F32, name=f"at_acc{q}",
                                        tag=f"at_acc{q}") for q in range(NQT)]
                    for mt in range(NMACRO // 2):
                        emit_A(0, mt, sp)
                    for qt in range(NQT):
                        emit_A(0, NMACRO // 2 + qt, sp)
                        if qt == NQT - 1:
                            # B(0,7) reads qT from the A-block just emitted,
                            # whose deferred transposes must land first
                            flush_drains()
                        emit_B(0, qt, 0, KC // 2 - 1, acc=at_acc[qt])
                    for qt in range(NQT):
                        emit_A(1, 2 * qt, sp)
                        emit_A(1, 2 * qt + 1, sp)
                        emit_B(0, qt, KC // 2, KC - 1, acc=at_acc[qt])
                    flush_drains()

                nc.gpsimd.collective_compute(
                    "AllToAll", ALU.bypass,
                    ins=[a2a_in[0][:].opt()], outs=[a2a_out[0][:].opt()],
                    replica_groups=[list(range(NCORES))])

                # ---- phase 3: batch-1 attention + batch-0 out projection ----
                with tc.tile_pool(name="cstage", bufs=1) as cp, \
                     tc.tile_pool(name="cwork", bufs=2) as cw:
                    wout_sb = cp.tile([128, 8, C], BF16)
                    nc.sync.dma_start(
                        wout_sb[:], wout_d.ap().rearrange("(a p) n -> p a n", p=128))
                    atf = cp.tile([128, 8, SHARD], BF16)
                    nc.sync.dma_start(atf[:, :, 0:512],
                                      a2a_out[0][:].transpose([1, 0, 2]))

                    def emit_C(ttk, half, ost):
                        # atomic unit: 16 matmuls in two bank-alternating
                        # half-chains + psum drain (shares pssc ring)
                        ps_o = pssc.tile([128, 1024], F32, name="ps_o", tag="pssc")
                        for cc in range(8):
                            lhs = atf[:, cc, ttk * 128:(ttk + 1) * 128]
                            nc.tensor.matmul(
                                ps_o[:, 0:256], lhsT=lhs,
                                rhs=wout_sb[:, cc, half * 512:half * 512 + 256],
                                start=(cc == 0), stop=(cc == 7))
                            nc.tensor.matmul(
                                ps_o[:, 512:768], lhsT=lhs,
                                rhs=wout_sb[:, cc, half * 512 + 256:
                                            half * 512 + 512],
                                start=(cc == 0), stop=(cc == 7))
                        nc.vector.tensor_copy(
                            ost[:, half * 512:(half + 1) * 512]
                            .rearrange("p (a b) -> p a b", a=2),
                            ps_o[:].rearrange("p (a b) -> p a b", a=2)[:, :, 0:256])
                        if half == 1:
                            nc.sync.dma_start(
                                out_d.ap()[ttk * 128:(ttk + 1) * 128, :], ost[:])

                    # all of batch-0's out projection is reserved to keep PE
                    # busy during the batch-1 AllToAll (which carries ~28us of
                    # cross-core skew on top of the transfer itself)
                    for qt in range(NQT):
                        emit_B(1, qt)
                    flush_drains()

                    nc.gpsimd.collective_compute(
                        "AllToAll", ALU.bypass,
                        ins=[a2a_in[1][:].opt()], outs=[a2a_out[1][:].opt()],
                        replica_groups=[list(range(NCORES))])

                    for ttk in range(4):
                        ost = cw.tile([128, C], F32, name="ostage", tag="ostage")
                        for half in range(2):
                            emit_C(ttk, half, ost)
                    nc.sync.dma_start(atf[:, :, 512:1024],
                                      a2a_out[1][:].transpose([1, 0, 2]))
                    for ttk in range(SHARD // 128 // 2, SHARD // 128):
                        ost = cw.tile([128, C], F32, name="ostage", tag="ostage")
                        for half in range(2):
                            emit_C(ttk, half, ost)

    nc.compile()
    return nc


def _fold_sin(sin, g):
    out = np.empty_like(sin)
    out[:, :32] = -sin[:, :32] * g[32:]
    out[:, 32:] = sin[:, 32:] * g[:32]
    return out


def kernel(hidden_states, cos, sin, Wqkv, Wout, gq, gk):
    global _LAST_RESULT
    _install_profile_shim()

    hidden_states = np.asarray(hidden_states, dtype=np.float32)
    cos = np.asarray(cos, dtype=np.float32)
    sin = np.asarray(sin, dtype=np.float32)
    Wqkv = np.asarray(Wqkv, dtype=np.float32)
    Wout = np.asarray(Wout, dtype=np.float32)
    gq = np.asarray(gq, dtype=np.float32)
    gk = np.asarray(gk, dtype=np.float32)

    if "nc" not in _CACHE:
        _CACHE["nc"] = _build_graph()
    nc = _CACHE["nc"]

    hsT = np.ascontiguousarray(hidden_states.reshape(TOK, C).T).astype(ml_dtypes.bfloat16)
    cosq = cos * gq[None, :]
    sinq = _fold_sin(sin, gq)
    cosk = cos * gk[None, :]
    sink = _fold_sin(sin, gk)
    trigc = np.concatenate([cosq, cosq, cosk, cosk], axis=1).astype(ml_dtypes.bfloat16)
    trigs = np.concatenate([sinq, sinq, sink, sink], axis=1).astype(ml_dtypes.bfloat16)
    wout_bf = Wout.astype(ml_dtypes.bfloat16)

    in_maps = []
    for c in range(NCORES):
        wq = Wqkv[:, c * 128:(c + 1) * 128]
        wk = Wqkv[:, C + c * 128:C + (c + 1) * 128]
        wv = Wqkv[:, 2 * C + c * 128:2 * C + (c + 1) * 128]
        wqkv_loc = np.ascontiguousarray(
            np.concatenate([wq, wk, wv], axis=1)).astype(ml_dtypes.bfloat16)
        in_maps.append({
            "hsT": hsT, "wqkv": wqkv_loc, "trigc": trigc, "trigs": trigs,
            "wout": wout_bf,
        })

    trace = bool(os.environ.get("BASS_TRACE"))
    res = run_bass_kernel_spmd(nc, in_maps, core_ids=list(range(NCORES)), trace=trace)
    _LAST_RESULT = res

    full = np.empty((B, N, C), dtype=np.float32)
    for c in range(NCORES):
        o = res.results[c]["out"]
        for b in range(B):
            full[b, c * 512:(c + 1) * 512, :] = o[b * 512:(b + 1) * 512]
    return full

